# revision 1
# baseline (speedup 1.0000x reference)
"""Hetero-GNN (3x GATv2) Trainium2 kernel.

Strategy (8 cores, full I/O):
  - dst-partition both node types across the 8 cores (6250 dst rows each).
  - Phase 1 (on device, replicated): hl_r = x_src @ Wl_r for each relation,
    stored in DRAM as fp32 rows [feat(128) | 1.0 | att.hl] (130 cols, 520B);
    hr_r for the core's own dst slice as [feat(128) | att.hr] (129 cols).
  - Phase 2: edges sorted by dst, dsts binned into 49 degree-balanced
    windows of 128 dst slots. Per 128-edge subchunk: indirect-DMA row
    gathers of hl[src] and hr[dst], z = g + h, leaky-relu via
    e = (att.g + att.h) + 0.8 * sum(att * relu(-z)), w = exp(e) (exact
    softmax without max-subtraction; logits are O(5) so exp is safe),
    one-hot weighted matrix S[k, d] = w_k * (slot_k == d) built with a
    single fused tensor_scalar, then TensorE matmul S^T @ [g | 1]
    accumulates numerator and denominator in PSUM over the window.
  - Window epilogue: out = relu(mean_r(acc / den)), written per dst slot;
    host inverts the window permutation and concatenates core slices.
No collectives needed: inputs replicated, outputs disjoint.
"""

import numpy as np
import ml_dtypes

import concourse.bass as bass
import concourse.tile as tile
from concourse import mybir
from concourse.bass_utils import run_bass_kernel_spmd

P = 128
NCORES = 8
N = 50000          # nodes per type
D = 128            # in feats
C = 128            # out feats
E = 600000         # edges per relation
ND = N // NCORES   # 6250 dst nodes per core
NW = 49            # windows per core (49*128 = 6272 >= 6250)
DSTPAD = NW * P    # 6272
NNP = 392 * P      # 50176 padded source-node count
HLW = 130          # hl row: 128 feats | 1.0 | att.hl
HRW = 129          # hr row: 128 feats | att.hr
SLOPE = 0.2
RELS = ("ab", "ba", "aa")
BF16 = mybir.dt.bfloat16
F32 = mybir.dt.float32
I32 = mybir.dt.int32

_BUILD_CACHE = {}


def _build_program(subs):
    """subs: dict rel -> subchunks-per-window (compile-time constants)."""
    nc = bass.Bass()

    # ---- I/O declarations ----
    inp = {}
    for nm, shape, dt in [
        ("xT_a", [P, NNP], BF16), ("xT_b", [P, NNP], BF16),
        ("xTd_a", [P, DSTPAD], BF16), ("xTd_b", [P, DSTPAD], BF16),
        ("iota", [P, P], F32),
    ]:
        inp[nm] = nc.dram_tensor(nm, shape, dt, kind="ExternalInput")
    for r in RELS:
        ns = NW * subs[r]
        for nm, shape, dt in [
            (f"wl_{r}", [P, HLW], BF16), (f"wr_{r}", [P, HRW], BF16),
            (f"att_{r}", [P, P], F32),
            (f"srcT_{r}", [P, ns], I32), (f"dstT_{r}", [P, ns], I32),
            (f"relT_{r}", [P, ns], F32),
        ]:
            inp[nm] = nc.dram_tensor(nm, shape, dt, kind="ExternalInput")

    out_a = nc.dram_tensor("out_a", [DSTPAD, C], F32, kind="ExternalOutput")
    out_b = nc.dram_tensor("out_b", [DSTPAD, C], F32, kind="ExternalOutput")

    hl = {r: nc.dram_tensor(f"hl_{r}", [NNP, HLW], F32) for r in RELS}
    hr = {r: nc.dram_tensor(f"hr_{r}", [DSTPAD, HRW], F32) for r in RELS}

    src_of = {"ab": "xT_a", "ba": "xT_b", "aa": "xT_a"}
    dst_of = {"ab": "xTd_b", "ba": "xTd_a", "aa": "xTd_a"}

    with tile.TileContext(nc) as tc:
        with (
            tc.tile_pool(name="consts", bufs=1) as consts,
            tc.tile_pool(name="xin", bufs=3) as xin,
            tc.tile_pool(name="p1ps", bufs=3, space="PSUM") as p1ps,
            tc.tile_pool(name="p1ep", bufs=3) as p1ep,
            tc.tile_pool(name="gath", bufs=3) as gath,
            tc.tile_pool(name="work", bufs=3) as work,
            tc.tile_pool(name="small", bufs=4) as small,
            tc.tile_pool(name="p2ps", bufs=4, space="PSUM") as p2ps,
            tc.tile_pool(name="outp", bufs=4) as outp,
        ):
            # ---- load constants ----
            iota_t = consts.tile([P, P], F32, tag="iota")
            nc.sync.dma_start(out=iota_t[:], in_=inp["iota"][:])
            wl_t, wr_t, att_t, srcT_t, dstT_t, relT_t = {}, {}, {}, {}, {}, {}
            for r in RELS:
                ns = NW * subs[r]
                wl_t[r] = consts.tile([P, HLW], BF16, tag=f"wl{r}", name=f"wl{r}")
                wr_t[r] = consts.tile([P, HRW], BF16, tag=f"wr{r}", name=f"wr{r}")
                att_t[r] = consts.tile([P, P], F32, tag=f"att{r}", name=f"att{r}")
                srcT_t[r] = consts.tile([P, ns], I32, tag=f"src{r}", name=f"src{r}")
                dstT_t[r] = consts.tile([P, ns], I32, tag=f"dst{r}", name=f"dst{r}")
                relT_t[r] = consts.tile([P, ns], F32, tag=f"rel{r}", name=f"rel{r}")
                for t, nm in [
                    (wl_t[r], f"wl_{r}"), (wr_t[r], f"wr_{r}"),
                    (att_t[r], f"att_{r}"), (srcT_t[r], f"srcT_{r}"),
                    (dstT_t[r], f"dstT_{r}"), (relT_t[r], f"relT_{r}"),
                ]:
                    nc.sync.dma_start(out=t[:], in_=inp[nm][:])
            xd_t = {}
            for nm in ("xTd_a", "xTd_b"):
                xd_t[nm] = consts.tile([P, DSTPAD], BF16, tag=nm, name=nm)
                nc.sync.dma_start(out=xd_t[nm][:], in_=inp[nm][:])

            # ---- phase 1: projections ----
            def emit_phase1(r):
                xsrc = inp[src_of[r]]
                # hl: 49 outer chunks x 8 subchunks of 128 nodes
                for j in range(NNP // 1024):
                    xt = xin.tile([P, 1024], BF16, tag="xchunk")
                    nc.gpsimd.dma_start(
                        out=xt[:], in_=xsrc[:, j * 1024:(j + 1) * 1024])
                    ep = p1ep.tile([P, 8 * HLW], F32, tag="hl_ep")
                    ep3 = ep[:].rearrange("p (s c) -> p s c", c=HLW)
                    for s in range(8):
                        ps = p1ps.tile([P, HLW], F32, tag="p1ps")
                        nc.tensor.matmul(
                            out=ps[:], lhsT=xt[:, s * P:(s + 1) * P],
                            rhs=wl_t[r][:], start=True, stop=True)
                        nc.scalar.copy(out=ep3[:, s, :], in_=ps[:])
                    nc.vector.memset(ep3[:, :, 128:129], 1.0)
                    nc.scalar.dma_start(
                        out=hl[r][j * 1024:(j + 1) * 1024, :].rearrange(
                            "(s p) c -> p s c", p=P),
                        in_=ep3[:, :, :])
                # hr: 49 chunks of 128 dst rows, batches of 8
                xd = xd_t[dst_of[r]]
                for g in range((NW + 7) // 8):
                    cnt = min(8, NW - g * 8)
                    ep = p1ep.tile([P, 8 * HRW], F32, tag="hr_ep")
                    ep3 = ep[:].rearrange("p (s c) -> p s c", c=HRW)
                    for s in range(cnt):
                        jj = g * 8 + s
                        ps = p1ps.tile([P, HLW], F32, tag="p1ps", name="hr_ps")[:, :HRW]
                        nc.tensor.matmul(
                            out=ps[:], lhsT=xd[:, jj * P:(jj + 1) * P],
                            rhs=wr_t[r][:], start=True, stop=True)
                        nc.scalar.copy(out=ep3[:, s, :], in_=ps[:])
                    nc.scalar.dma_start(
                        out=hr[r][g * 1024:g * 1024 + cnt * P, :].rearrange(
                            "(s p) c -> p s c", p=P),
                        in_=ep3[:, :cnt, :])

            for r in RELS:
                emit_phase1(r)

            # ---- phase 2: edge processing, window-major ----
            def emit_window_rel(r, w):
                SUB = subs[r]
                i0 = w * SUB
                # gathers
                gt = gath.tile([P, SUB * HLW], F32, tag="G")
                ht = gath.tile([P, SUB * HRW], F32, tag="H")
                for s in range(SUB):
                    nc.gpsimd.indirect_dma_start(
                        out=gt[:, s * HLW:(s + 1) * HLW], out_offset=None,
                        in_=hl[r][:],
                        in_offset=bass.IndirectOffsetOnAxis(
                            ap=srcT_t[r][:, i0 + s:i0 + s + 1], axis=0))
                    nc.gpsimd.indirect_dma_start(
                        out=ht[:, s * HRW:(s + 1) * HRW], out_offset=None,
                        in_=hr[r][:],
                        in_offset=bass.IndirectOffsetOnAxis(
                            ap=dstT_t[r][:, i0 + s:i0 + s + 1], axis=0))
                g3 = gt[:].rearrange("p (s c) -> p s c", c=HLW)
                h3 = ht[:].rearrange("p (s c) -> p s c", c=HRW)
                # z = g + h (feat cols), sdot = att.g + att.h
                zt = work.tile([P, SUB * P], F32, tag="z")
                z3 = zt[:].rearrange("p (s c) -> p s c", c=P)
                nc.vector.tensor_tensor(
                    out=z3[:, :, :], in0=g3[:, :, 0:P], in1=h3[:, :, 0:P],
                    op=mybir.AluOpType.add)
                sdot = small.tile([P, SUB], F32, tag="sdot")
                nc.vector.tensor_tensor(
                    out=sdot[:].rearrange("p (s c) -> p s c", c=1),
                    in0=g3[:, :, 129:130], in1=h3[:, :, 128:129],
                    op=mybir.AluOpType.add)
                # r = relu(-z)
                rt = work.tile([P, SUB * P], F32, tag="rneg")
                nc.scalar.activation(
                    out=rt[:], in_=zt[:],
                    func=mybir.ActivationFunctionType.Relu, scale=-1.0)
                # value-path bf16 copy of [feat | 1] cols
                gb = work.tile([P, SUB * HRW], BF16, tag="gb16")
                nc.scalar.copy(
                    out=gb[:].rearrange("p (s c) -> p s c", c=HRW),
                    in_=g3[:, :, 0:HRW])
                # racc[s] = sum(att * r) per subchunk
                racc = small.tile([P, SUB], F32, tag="racc")
                for s in range(SUB):
                    ttrd = work.tile([P, P], F32, tag="ttrd", name="ttrd")
                    nc.vector.tensor_tensor(
                        out=ttrd[:], in0=rt[:, s * P:(s + 1) * P],
                        in1=att_t[r][:], op=mybir.AluOpType.mult)
                    nc.vector.tensor_reduce(
                        out=racc[:, s:s + 1], in_=ttrd[:],
                        axis=mybir.AxisListType.X, op=mybir.AluOpType.add)
                # e = sdot - 0.8 * racc ; w = exp(e)
                et = small.tile([P, SUB], F32, tag="e")
                nc.vector.tensor_scalar(
                    out=et[:], in0=racc[:], scalar1=(1.0 - SLOPE),
                    scalar2=None, op0=mybir.AluOpType.mult)
                nc.vector.tensor_tensor(
                    out=et[:], in0=et[:], in1=sdot[:],
                    op=mybir.AluOpType.add)
                wt = small.tile([P, SUB], F32, tag="w")
                nc.scalar.activation(
                    out=wt[:], in_=et[:],
                    func=mybir.ActivationFunctionType.Exp)
                # S[k, d] = w_k * (slot_k == d); matmul accumulate
                st = work.tile([P, SUB * P], BF16, tag="S")
                ps = p2ps.tile([P, HRW], F32, tag="acc")
                for s in range(SUB):
                    nc.vector.tensor_scalar(
                        out=st[:, s * P:(s + 1) * P], in0=iota_t[:],
                        scalar1=relT_t[r][:, i0 + s:i0 + s + 1],
                        scalar2=wt[:, s:s + 1],
                        op0=mybir.AluOpType.is_equal,
                        op1=mybir.AluOpType.mult)
                    nc.tensor.matmul(
                        out=ps[:], lhsT=st[:, s * P:(s + 1) * P],
                        rhs=gb[:, s * HRW:(s + 1) * HRW],
                        start=(s == 0), stop=(s == SUB - 1))
                # normalize: o = acc / (den + eps)
                den = small.tile([P, 1], F32, tag="den")
                nc.vector.tensor_scalar(
                    out=den[:], in0=ps[:, 128:129], scalar1=1e-12,
                    scalar2=None, op0=mybir.AluOpType.add)
                rcp = small.tile([P, 1], F32, tag="rcp")
                nc.vector.reciprocal(out=rcp[:], in_=den[:])
                ot = outp.tile([P, P], F32, tag=f"o_{r}")
                nc.vector.tensor_scalar(
                    out=ot[:], in0=ps[:, 0:P], scalar1=rcp[:],
                    scalar2=None, op0=mybir.AluOpType.mult)
                return ot

            for w in range(NW):
                # relation ab -> out_b
                o_ab = emit_window_rel("ab", w)
                ob = outp.tile([P, P], F32, tag="outb")
                nc.scalar.activation(
                    out=ob[:], in_=o_ab[:],
                    func=mybir.ActivationFunctionType.Relu)
                nc.sync.dma_start(
                    out=out_b[w * P:(w + 1) * P, :], in_=ob[:])
                # relations ba, aa -> out_a
                o_ba = emit_window_rel("ba", w)
                o_aa = emit_window_rel("aa", w)
                nc.vector.tensor_tensor(
                    out=o_ba[:], in0=o_ba[:], in1=o_aa[:],
                    op=mybir.AluOpType.add)
                oa = outp.tile([P, P], F32, tag="outa")
                nc.scalar.activation(
                    out=oa[:], in_=o_ba[:],
                    func=mybir.ActivationFunctionType.Relu, scale=0.5)
                nc.sync.dma_start(
                    out=out_a[w * P:(w + 1) * P, :], in_=oa[:])

    _spill_dma_waits(nc)
    return nc


def _spill_dma_waits(nc):
    """The bundled walrus build only accepts one embedded sync-wait per DMA
    pseudo-instruction. Move multi-waits onto a NoOp on the issuing engine
    (engines decode in order, so the DMA stays gated)."""
    for bbb in nc.bb_map.values():
        insts = bbb.bb.instructions
        out = []
        for ins in insts:
            si = getattr(ins, "sync_info", None)
            ow = list(si.on_wait) if si is not None and si.on_wait else []
            if len(ow) >= 2:
                for w in ow:
                    nop = mybir.InstNoOp(
                        name=nc.get_next_instruction_name(), ins=[], outs=[],
                        engine=ins.engine)
                    nop.sync_info = mybir.SyncInfo(on_wait=[w], on_update=[])
                    out.append(nop)
                ins.sync_info = mybir.SyncInfo(
                    on_wait=[], on_update=list(si.on_update or []))
            out.append(ins)
        insts[:] = out


# ---------------- host-side preprocessing ----------------

def _balanced_windows(deg):
    """Assign ND dsts to NW bins of <=128 slots, balancing total degree.
    Serpentine assignment over degree-sorted dsts. Returns win[d], slot[d]."""
    order = np.argsort(-deg, kind="stable")
    win = np.empty(ND, np.int32)
    slot = np.empty(ND, np.int32)
    fill = np.zeros(NW, np.int32)
    b = 0
    direction = 1
    for i, d in enumerate(order):
        # serpentine over bins, skipping full ones
        tries = 0
        while fill[b] >= P:
            b += direction
            if b == NW or b < 0:
                direction = -direction
                b += direction
            tries += 1
            assert tries <= 2 * NW
        win[d] = b
        slot[d] = fill[b]
        fill[b] += 1
        b += direction
        if b == NW or b < 0:
            direction = -direction
            b += direction
    return win, slot


def _pack_edges(src, dst_local, win, slot, sub):
    """Group edges by window, pad each window to sub*128 slots.
    Returns srcT, dstT, relT transposed [128, NW*sub] arrays."""
    ewin = win[dst_local]
    order = np.argsort(ewin, kind="stable")
    src_s = src[order]
    dstl_s = dst_local[order]
    erel_s = slot[dst_local][order].astype(np.float32)
    ewin_s = ewin[order]
    counts = np.bincount(ewin_s, minlength=NW)
    offs = np.zeros(NW + 1, np.int64)
    np.cumsum(counts, out=offs[1:])
    pos = np.arange(len(src_s)) - offs[ewin_s]
    flat = ewin_s.astype(np.int64) * (sub * P) + pos
    nslots = NW * sub * P
    srcp = np.zeros(nslots, np.int32)
    dstp = np.zeros(nslots, np.int32)
    relp = np.full(nslots, 999.0, np.float32)
    srcp[flat] = src_s
    dstp[flat] = dstl_s
    relp[flat] = erel_s
    to_T = lambda a: np.ascontiguousarray(a.reshape(NW * sub, P).T)
    return to_T(srcp), to_T(dstp), to_T(relp)


def kernel(**inputs):
    x_a = np.asarray(inputs["x_a"], np.float32)
    x_b = np.asarray(inputs["x_b"], np.float32)
    edges = {r: np.asarray(inputs[f"edge_{r}"]).astype(np.int64) for r in RELS}

    # shared device inputs
    def padT(x, cols):
        out = np.zeros((P, cols), ml_dtypes.bfloat16)
        out[:, :x.shape[0]] = x.T.astype(ml_dtypes.bfloat16)
        return out

    shared = {
        "xT_a": padT(x_a, NNP),
        "xT_b": padT(x_b, NNP),
        "iota": np.broadcast_to(
            np.arange(P, dtype=np.float32), (P, P)).copy(),
    }
    for r in RELS:
        Wl = np.asarray(inputs[f"Wl_{r}"], np.float32)
        Wr = np.asarray(inputs[f"Wr_{r}"], np.float32)
        att = np.asarray(inputs[f"att_{r}"], np.float32)
        for nm in ("bl", "br", "bias"):
            assert not np.any(np.asarray(inputs[f"{nm}_{r}"])), \
                f"nonzero {nm}_{r} not supported"
        wl = np.zeros((P, HLW), np.float32)
        wl[:, :C] = Wl
        wl[:, 129] = Wl @ att
        wr = np.zeros((P, HRW), np.float32)
        wr[:, :C] = Wr
        wr[:, 128] = Wr @ att
        shared[f"wl_{r}"] = wl.astype(ml_dtypes.bfloat16)
        shared[f"wr_{r}"] = wr.astype(ml_dtypes.bfloat16)
        shared[f"att_{r}"] = np.broadcast_to(att, (P, P)).copy()

    # per-core graph structure
    dst_type = {"ab": "b", "ba": "a", "aa": "a"}
    # sort edges by dst once per relation
    sorted_e = {}
    for r in RELS:
        s, d = edges[r][0], edges[r][1]
        o = np.argsort(d, kind="stable")
        sorted_e[r] = (s[o].astype(np.int32), d[o].astype(np.int32))

    core_data = []
    for c in range(NCORES):
        base = c * ND
        # combined degree per dst type for window balance
        deg = {"a": np.zeros(ND, np.int64), "b": np.zeros(ND, np.int64)}
        loc = {}
        for r in RELS:
            s, d = sorted_e[r]
            lo, hi = np.searchsorted(d, [base, base + ND])
            dl = (d[lo:hi] - base).astype(np.int64)
            loc[r] = (s[lo:hi], dl)
            deg[dst_type[r]] += np.bincount(dl, minlength=ND)
        winslot = {t: _balanced_windows(deg[t]) for t in ("a", "b")}
        core_data.append((loc, winslot))

    # global SUB per relation
    subs = {}
    for r in RELS:
        mx = 0
        for loc, winslot in core_data:
            win, _ = winslot[dst_type[r]]
            s, dl = loc[r]
            wc = np.bincount(win[dl], minlength=NW)
            mx = max(mx, int(wc.max()))
        subs[r] = max(1, -(-mx // P))

    key = tuple(sorted(subs.items()))
    if key not in _BUILD_CACHE:
        _BUILD_CACHE[key] = _build_program(subs)
    nc = _BUILD_CACHE[key]

    in_maps = []
    for c in range(NCORES):
        base = c * ND
        loc, winslot = core_data[c]
        m = dict(shared)

        def dslice(x):
            sl = np.zeros((DSTPAD, D), np.float32)
            end = min(N, base + DSTPAD)
            sl[:end - base] = x[base:end]
            return np.ascontiguousarray(sl.T).astype(ml_dtypes.bfloat16)

        m["xTd_a"] = dslice(x_a)
        m["xTd_b"] = dslice(x_b)
        for r in RELS:
            win, slot = winslot[dst_type[r]]
            s, dl = loc[r]
            srcT, dstT, relT = _pack_edges(s, dl, win, slot, subs[r])
            m[f"srcT_{r}"] = srcT
            m[f"dstT_{r}"] = dstT
            m[f"relT_{r}"] = relT
        in_maps.append(m)

    res = run_bass_kernel_spmd(nc, in_maps, core_ids=list(range(NCORES)))

    out_a = np.empty((N, C), np.float32)
    out_b = np.empty((N, C), np.float32)
    for c in range(NCORES):
        base = c * ND
        _, winslot = core_data[c]
        for t, full in (("a", out_a), ("b", out_b)):
            win, slot = winslot[t]
            rowmap = win.astype(np.int64) * P + slot
            dev = res.results[c][f"out_{t}"]
            full[base:base + ND] = dev[rowmap]
    return out_a, out_b



# revision 5
# speedup vs baseline: 1.9054x; 1.9054x over previous
"""Hetero-GNN (3x GATv2) Trainium2 kernel.

The run is dominated by host<->device transfer through the tunnel
(~40 MB/s), so the layout is built to minimize bytes moved per call:

  - Each core uploads only its own 6272-row dst slice of x_a|x_b
    ([256, 6272] bf16, feature-major); a device AllGather across the 8
    cores rebuilds the full feature matrix xg, from which each core
    computes the replicated source projections hl_r = x_src @ Wl_r
    (rows [feat(128) | 1.0 | att.hl], fp32) and its own dst projections
    hr_r ([feat(128) | att.hr]).
  - dst ownership is the natural range [c*6272, (c+1)*6272); windows are
    contiguous 128-dst blocks, so the one-hot slot id is derived on
    device as (iota + 128*w == dst_local) -- no slot array upload and no
    output permutation.
  - Edge endpoints are uploaded as uint16 (node ids < 65536), widened to
    int32/f32 on device. Pad slots point src at row 0 and dst at the
    sentinel row 6272 (hr has 128 zeroed extra rows); the sentinel never
    matches the slot-iota so padded edges contribute exactly zero.
  - Per 128-edge subchunk: indirect-DMA row gathers of hl[src] and
    hr[dst], z = g + h, e = (att.g + att.h) + 0.8 * sum(att * relu(-z)),
    w = exp(e) (exact softmax without max-subtraction; logits are O(10)
    so fp32 exp is safe), S[k, d] = w_k * (iota_w == dst_k) built with a
    single fused tensor_scalar, then TensorE matmul S^T @ [feat | 1]
    accumulates numerator and denominator in PSUM over the window.
  - Window epilogue: out = relu(mean_r(acc / den)) written as fp16 at
    the natural dst offset; host concatenates core slices.
"""

import numpy as np
import ml_dtypes

import concourse.bass as bass
import concourse.tile as tile
from concourse import mybir
from concourse.bass_utils import run_bass_kernel_spmd

P = 128
NCORES = 8
N = 50000          # nodes per type
D = 128            # in feats
C = 128            # out feats
E = 600000         # edges per relation
NW = 49            # windows per core
NDC = NW * P       # 6272 dst slots per core per type; 8*6272 = 50176 >= N
NNP = NCORES * NDC # 50176 padded node count (hl table rows)
HLW = 130          # hl row: 128 feats | 1.0 | att.hl
HRW = 129          # hr row: 128 feats | att.hr
HRROWS = NDC + P   # 6400: +128 zeroed sentinel rows
SENT = NDC         # sentinel dst index for pad slots
SLOPE = 0.2
RELS = ("ab", "ba", "aa")
BF16 = mybir.dt.bfloat16
F32 = mybir.dt.float32
F16 = mybir.dt.float16
I32 = mybir.dt.int32
U16 = mybir.dt.uint16

_BUILD_CACHE = {}


def _build_program(subs):
    """subs: dict rel -> subchunks-per-window (compile-time constants)."""
    nc = bass.Bass()

    # ---- I/O declarations ----
    inp = {"xd": nc.dram_tensor("xd", [2 * P, NDC], BF16, kind="ExternalInput")}
    for r in RELS:
        ns = NW * subs[r]
        for nm, shape, dt in [
            (f"wl_{r}", [P, HLW], BF16), (f"wr_{r}", [P, HRW], BF16),
            (f"attv_{r}", [1, P], BF16),
            (f"srcT_{r}", [P, ns], U16), (f"dstT_{r}", [P, ns], U16),
        ]:
            inp[nm] = nc.dram_tensor(nm, shape, dt, kind="ExternalInput")

    out_a = nc.dram_tensor("out_a", [NDC, C], F16, kind="ExternalOutput")
    out_b = nc.dram_tensor("out_b", [NDC, C], F16, kind="ExternalOutput")

    hl = {r: nc.dram_tensor(f"hl_{r}", [NNP, HLW], F32) for r in RELS}
    hr = {r: nc.dram_tensor(f"hr_{r}", [HRROWS, HRW], F32) for r in RELS}

    # xg block layout: [core(8)][type(2)][feat(128)] x [6272 cols]
    src_toff = {"ab": 0, "ba": P, "aa": 0}      # src type row offset in xg blocks
    dst_roff = {"ab": P, "ba": 0, "aa": 0}      # dst type row offset in xd

    with tile.TileContext(nc) as tc:
        with (
            tc.tile_pool(name="dram", bufs=1, space="DRAM") as dram,
            tc.tile_pool(name="consts", bufs=1) as consts,
            tc.tile_pool(name="xin", bufs=3) as xin,
            tc.tile_pool(name="p1ps", bufs=3, space="PSUM") as p1ps,
            tc.tile_pool(name="p1ep", bufs=3) as p1ep,
            tc.tile_pool(name="gath", bufs=2) as gath,
            tc.tile_pool(name="work", bufs=2) as work,
            tc.tile_pool(name="small", bufs=4) as small,
            tc.tile_pool(name="p2ps", bufs=4, space="PSUM") as p2ps,
            tc.tile_pool(name="outp", bufs=4) as outp,
        ):
            # ---- x all-gather: per-core dst slice -> full feature matrix ----
            bounce = dram.tile([2 * P, NDC], BF16, tag="bounce")
            xg = dram.tile([NCORES * 2 * P, NDC], BF16, tag="xg")
            nc.gpsimd.dma_start(out=bounce[:], in_=inp["xd"][:])
            nc.gpsimd.collective_compute(
                "AllGather", mybir.AluOpType.bypass,
                replica_groups=[list(range(NCORES))],
                ins=[bounce[:].opt()], outs=[xg[:].opt()],
            )

            # ---- constants ----
            iota_i = consts.tile([P, P], I32, tag="iota_i")
            nc.gpsimd.iota(iota_i[:], [[1, P]], base=0, channel_multiplier=0)
            iota_t = consts.tile([P, P], F32, tag="iota")
            nc.scalar.copy(out=iota_t[:], in_=iota_i[:])
            ones1 = consts.tile([1, P], BF16, tag="ones1")
            nc.vector.memset(ones1[:], 1.0)

            wl_t, wr_t, att_t, src32, dst32, dstf = {}, {}, {}, {}, {}, {}
            for r in RELS:
                ns = NW * subs[r]
                wl_t[r] = consts.tile([P, HLW], BF16, tag=f"wl{r}", name=f"wl{r}")
                wr_t[r] = consts.tile([P, HRW], BF16, tag=f"wr{r}", name=f"wr{r}")
                attv = consts.tile([1, P], BF16, tag=f"attv{r}")
                su = consts.tile([P, ns], U16, tag=f"su{r}")
                du = consts.tile([P, ns], U16, tag=f"du{r}")
                for t, nm in [
                    (wl_t[r], f"wl_{r}"), (wr_t[r], f"wr_{r}"),
                    (attv, f"attv_{r}"), (su, f"srcT_{r}"), (du, f"dstT_{r}"),
                ]:
                    nc.sync.dma_start(out=t[:], in_=inp[nm][:])
                # broadcast att row to all 128 partitions via K=1 matmul
                aps = p1ps.tile([P, HLW], F32, tag="p1ps", name=f"attps{r}")
                nc.tensor.matmul(out=aps[:, :P], lhsT=ones1[:], rhs=attv[:],
                                 start=True, stop=True)
                att_t[r] = consts.tile([P, P], F32, tag=f"att{r}", name=f"att{r}")
                nc.scalar.copy(out=att_t[r][:], in_=aps[:, :P])
                # widen edge endpoints
                src32[r] = consts.tile([P, ns], I32, tag=f"s32{r}", name=f"s32{r}")
                nc.scalar.copy(out=src32[r][:], in_=su[:])
                dst32[r] = consts.tile([P, ns], I32, tag=f"d32{r}", name=f"d32{r}")
                nc.scalar.copy(out=dst32[r][:], in_=du[:])
                dstf[r] = consts.tile([P, ns], F32, tag=f"df{r}", name=f"df{r}")
                nc.scalar.copy(out=dstf[r][:], in_=dst32[r][:])

            # own dst x slices (straight from the ExternalInput)
            xda = consts.tile([P, NDC], BF16, tag="xda")
            nc.sync.dma_start(out=xda[:], in_=inp["xd"][0:P, :])
            xdb = consts.tile([P, NDC], BF16, tag="xdb")
            nc.sync.dma_start(out=xdb[:], in_=inp["xd"][P:2 * P, :])

            # zero the 128 sentinel rows of each hr table
            zt0 = consts.tile([P, HRW], F32, tag="zt0")
            nc.vector.memset(zt0[:], 0.0)
            for r in RELS:
                nc.sync.dma_start(out=hr[r][NDC:HRROWS, :], in_=zt0[:])

            # ---- phase 1: projections ----
            def emit_phase1(r):
                toff = src_toff[r]
                # hl: 8 gathered blocks x 7 chunks of 896 source nodes
                for g in range(NCORES):
                    for cb in range(7):
                        xt = xin.tile([P, 896], BF16, tag="xchunk")
                        nc.gpsimd.dma_start(
                            out=xt[:],
                            in_=xg[g * 2 * P + toff:g * 2 * P + toff + P,
                                   cb * 896:(cb + 1) * 896])
                        ep = p1ep.tile([P, 7 * HLW], F32, tag="hl_ep")
                        ep3 = ep[:].rearrange("p (s c) -> p s c", c=HLW)
                        for s in range(7):
                            ps = p1ps.tile([P, HLW], F32, tag="p1ps")
                            nc.tensor.matmul(
                                out=ps[:], lhsT=xt[:, s * P:(s + 1) * P],
                                rhs=wl_t[r][:], start=True, stop=True)
                            nc.scalar.copy(out=ep3[:, s, :], in_=ps[:])
                        nc.vector.memset(ep3[:, :, 128:129], 1.0)
                        nc.scalar.dma_start(
                            out=hl[r][g * NDC + cb * 896:
                                      g * NDC + (cb + 1) * 896, :].rearrange(
                                "(s p) c -> p s c", p=P),
                            in_=ep3[:, :, :])
                # hr: 49 windows of the core's own dst slice, batches of 7
                xdt = xda if dst_roff[r] == 0 else xdb
                for b in range(7):
                    ep = p1ep.tile([P, 7 * HRW], F32, tag="hr_ep")
                    ep3 = ep[:].rearrange("p (s c) -> p s c", c=HRW)
                    for s in range(7):
                        w = b * 7 + s
                        ps = p1ps.tile([P, HLW], F32, tag="p1ps",
                                       name="hr_ps")[:, :HRW]
                        nc.tensor.matmul(
                            out=ps[:], lhsT=xdt[:, w * P:(w + 1) * P],
                            rhs=wr_t[r][:], start=True, stop=True)
                        nc.scalar.copy(out=ep3[:, s, :], in_=ps[:])
                    nc.scalar.dma_start(
                        out=hr[r][b * 896:(b + 1) * 896, :].rearrange(
                            "(s p) c -> p s c", p=P),
                        in_=ep3[:, :, :])

            for r in RELS:
                emit_phase1(r)

            # ---- phase 2: edge processing, window-major ----
            def emit_window_rel(r, w, iw):
                SUB = subs[r]
                i0 = w * SUB
                # gathers
                gt = gath.tile([P, SUB * HLW], F32, tag="G")
                ht = gath.tile([P, SUB * HRW], F32, tag="H")
                for s in range(SUB):
                    nc.gpsimd.indirect_dma_start(
                        out=gt[:, s * HLW:(s + 1) * HLW], out_offset=None,
                        in_=hl[r][:],
                        in_offset=bass.IndirectOffsetOnAxis(
                            ap=src32[r][:, i0 + s:i0 + s + 1], axis=0))
                    nc.gpsimd.indirect_dma_start(
                        out=ht[:, s * HRW:(s + 1) * HRW], out_offset=None,
                        in_=hr[r][:],
                        in_offset=bass.IndirectOffsetOnAxis(
                            ap=dst32[r][:, i0 + s:i0 + s + 1], axis=0))
                g3 = gt[:].rearrange("p (s c) -> p s c", c=HLW)
                h3 = ht[:].rearrange("p (s c) -> p s c", c=HRW)
                # z = g + h (feat cols), sdot = att.g + att.h
                zt = work.tile([P, SUB * P], F32, tag="z")
                z3 = zt[:].rearrange("p (s c) -> p s c", c=P)
                nc.vector.tensor_tensor(
                    out=z3[:, :, :], in0=g3[:, :, 0:P], in1=h3[:, :, 0:P],
                    op=mybir.AluOpType.add)
                sdot = small.tile([P, SUB], F32, tag="sdot")
                nc.vector.tensor_tensor(
                    out=sdot[:].rearrange("p (s c) -> p s c", c=1),
                    in0=g3[:, :, 129:130], in1=h3[:, :, 128:129],
                    op=mybir.AluOpType.add)
                # rneg = relu(-z)
                rt = work.tile([P, SUB * P], F32, tag="rneg")
                nc.scalar.activation(
                    out=rt[:], in_=zt[:],
                    func=mybir.ActivationFunctionType.Relu, scale=-1.0)
                # value-path bf16 copy of [feat | 1] cols
                gb = work.tile([P, SUB * HRW], BF16, tag="gb16")
                nc.scalar.copy(
                    out=gb[:].rearrange("p (s c) -> p s c", c=HRW),
                    in_=g3[:, :, 0:HRW])
                # racc[s] = sum(att * rneg) per subchunk
                racc = small.tile([P, SUB], F32, tag="racc")
                for s in range(SUB):
                    ttrd = work.tile([P, P], F32, tag="ttrd", name="ttrd")
                    nc.vector.tensor_tensor(
                        out=ttrd[:], in0=rt[:, s * P:(s + 1) * P],
                        in1=att_t[r][:], op=mybir.AluOpType.mult)
                    nc.vector.tensor_reduce(
                        out=racc[:, s:s + 1], in_=ttrd[:],
                        axis=mybir.AxisListType.X, op=mybir.AluOpType.add)
                # e = sdot + 0.8 * racc  (racc holds att.relu(-z), i.e. the
                # negative part; adding 0.8 of it back yields att.leaky(z))
                et = small.tile([P, SUB], F32, tag="e")
                nc.vector.tensor_scalar(
                    out=et[:], in0=racc[:], scalar1=(1.0 - SLOPE),
                    scalar2=None, op0=mybir.AluOpType.mult)
                nc.vector.tensor_tensor(
                    out=et[:], in0=et[:], in1=sdot[:],
                    op=mybir.AluOpType.add)
                wt = small.tile([P, SUB], F32, tag="w")
                nc.scalar.activation(
                    out=wt[:], in_=et[:],
                    func=mybir.ActivationFunctionType.Exp)
                # S[k, d] = w_k * (iota_w == dst_k); matmul accumulate
                st = work.tile([P, SUB * P], BF16, tag="S")
                ps = p2ps.tile([P, HRW], F32, tag="acc")
                for s in range(SUB):
                    nc.vector.tensor_scalar(
                        out=st[:, s * P:(s + 1) * P], in0=iw[:],
                        scalar1=dstf[r][:, i0 + s:i0 + s + 1],
                        scalar2=wt[:, s:s + 1],
                        op0=mybir.AluOpType.is_equal,
                        op1=mybir.AluOpType.mult)
                    nc.tensor.matmul(
                        out=ps[:], lhsT=st[:, s * P:(s + 1) * P],
                        rhs=gb[:, s * HRW:(s + 1) * HRW],
                        start=(s == 0), stop=(s == SUB - 1))
                # normalize: o = acc / (den + eps)
                den = small.tile([P, 1], F32, tag="den")
                nc.vector.tensor_scalar(
                    out=den[:], in0=ps[:, 128:129], scalar1=1e-12,
                    scalar2=None, op0=mybir.AluOpType.add)
                rcp = small.tile([P, 1], F32, tag="rcp")
                nc.vector.reciprocal(out=rcp[:], in_=den[:])
                ot = outp.tile([P, P], F32, tag=f"o_{r}")
                nc.vector.tensor_scalar(
                    out=ot[:], in0=ps[:, 0:P], scalar1=rcp[:],
                    scalar2=None, op0=mybir.AluOpType.mult)
                return ot

            for w in range(NW):
                iw = small.tile([P, P], F32, tag="iw")
                nc.vector.tensor_scalar(
                    out=iw[:], in0=iota_t[:], scalar1=float(w * P),
                    scalar2=None, op0=mybir.AluOpType.add)
                # relation ab -> out_b
                o_ab = emit_window_rel("ab", w, iw)
                ob = outp.tile([P, C], F16, tag="outb")
                nc.scalar.activation(
                    out=ob[:], in_=o_ab[:],
                    func=mybir.ActivationFunctionType.Relu)
                nc.sync.dma_start(
                    out=out_b[w * P:(w + 1) * P, :], in_=ob[:])
                # relations ba, aa -> out_a
                o_ba = emit_window_rel("ba", w, iw)
                o_aa = emit_window_rel("aa", w, iw)
                nc.vector.tensor_tensor(
                    out=o_ba[:], in0=o_ba[:], in1=o_aa[:],
                    op=mybir.AluOpType.add)
                oa = outp.tile([P, C], F16, tag="outa")
                nc.scalar.activation(
                    out=oa[:], in_=o_ba[:],
                    func=mybir.ActivationFunctionType.Relu, scale=0.5)
                nc.sync.dma_start(
                    out=out_a[w * P:(w + 1) * P, :], in_=oa[:])

    _spill_dma_waits(nc)
    return nc


def _spill_dma_waits(nc):
    """The bundled walrus build only accepts one embedded sync-wait per
    pseudo-instruction. Move multi-waits onto a NoOp on the issuing engine
    (engines decode in order, so the instruction stays gated)."""
    for bbb in nc.bb_map.values():
        insts = bbb.bb.instructions
        out = []
        for ins in insts:
            si = getattr(ins, "sync_info", None)
            ow = list(si.on_wait) if si is not None and si.on_wait else []
            if len(ow) >= 2:
                for w in ow:
                    nop = mybir.InstNoOp(
                        name=nc.get_next_instruction_name(), ins=[], outs=[],
                        engine=ins.engine)
                    nop.sync_info = mybir.SyncInfo(on_wait=[w], on_update=[])
                    out.append(nop)
                ins.sync_info = mybir.SyncInfo(
                    on_wait=[], on_update=list(si.on_update or []))
            out.append(ins)
        insts[:] = out


# ---------------- host-side preprocessing ----------------

def _pack_edges(src, dl, sub):
    """Edges of one core (sorted by local dst dl), windows = dl >> 7.
    Returns srcT, dstT transposed [128, NW*sub] uint16 arrays."""
    win = dl >> 7
    counts = np.bincount(win, minlength=NW)
    offs = np.zeros(NW + 1, np.int64)
    np.cumsum(counts, out=offs[1:])
    pos = np.arange(len(dl), dtype=np.int64) - offs[win]
    flat = win.astype(np.int64) * (sub * P) + pos
    nslots = NW * sub * P
    srcp = np.zeros(nslots, np.uint16)
    dstp = np.full(nslots, SENT, np.uint16)
    srcp[flat] = src.astype(np.uint16)
    dstp[flat] = dl.astype(np.uint16)
    to_T = lambda a: np.ascontiguousarray(a.reshape(NW * sub, P).T)
    return to_T(srcp), to_T(dstp)


def kernel(**inputs):
    x_a = np.asarray(inputs["x_a"], np.float32)
    x_b = np.asarray(inputs["x_b"], np.float32)
    edges = {r: np.asarray(inputs[f"edge_{r}"]).astype(np.int64) for r in RELS}

    shared = {}
    for r in RELS:
        Wl = np.asarray(inputs[f"Wl_{r}"], np.float32)
        Wr = np.asarray(inputs[f"Wr_{r}"], np.float32)
        att = np.asarray(inputs[f"att_{r}"], np.float32)
        for nm in ("bl", "br", "bias"):
            assert not np.any(np.asarray(inputs[f"{nm}_{r}"])), \
                f"nonzero {nm}_{r} not supported"
        wl = np.zeros((P, HLW), np.float32)
        wl[:, :C] = Wl
        wl[:, 129] = Wl @ att
        wr = np.zeros((P, HRW), np.float32)
        wr[:, :C] = Wr
        wr[:, 128] = Wr @ att
        shared[f"wl_{r}"] = wl.astype(ml_dtypes.bfloat16)
        shared[f"wr_{r}"] = wr.astype(ml_dtypes.bfloat16)
        shared[f"attv_{r}"] = att[None, :].astype(ml_dtypes.bfloat16)

    # sort edges by dst once per relation
    sorted_e = {}
    for r in RELS:
        s, d = edges[r][0], edges[r][1]
        o = np.argsort(d, kind="stable")
        sorted_e[r] = (s[o], d[o])

    # global subchunks-per-window per relation (windows are natural
    # 128-dst blocks: global window id of dst d is d >> 7)
    subs = {}
    for r in RELS:
        wc = np.bincount(sorted_e[r][1] >> 7, minlength=NCORES * NW)
        subs[r] = max(1, -(-int(wc.max()) // P))

    key = tuple(sorted(subs.items()))
    if key not in _BUILD_CACHE:
        _BUILD_CACHE[key] = _build_program(subs)
    nc = _BUILD_CACHE[key]

    in_maps = []
    for c in range(NCORES):
        base = c * NDC
        cnt = min(NDC, N - base)
        m = dict(shared)
        xd = np.zeros((2 * P, NDC), ml_dtypes.bfloat16)
        xd[0:P, :cnt] = x_a[base:base + cnt].T.astype(ml_dtypes.bfloat16)
        xd[P:2 * P, :cnt] = x_b[base:base + cnt].T.astype(ml_dtypes.bfloat16)
        m["xd"] = xd
        for r in RELS:
            s, d = sorted_e[r]
            lo, hi = np.searchsorted(d, [base, base + NDC])
            srcT, dstT = _pack_edges(s[lo:hi], d[lo:hi] - base, subs[r])
            m[f"srcT_{r}"] = srcT
            m[f"dstT_{r}"] = dstT
        in_maps.append(m)

    res = run_bass_kernel_spmd(nc, in_maps, core_ids=list(range(NCORES)))

    out_a = np.empty((N, C), np.float32)
    out_b = np.empty((N, C), np.float32)
    for c in range(NCORES):
        base = c * NDC
        cnt = min(NDC, N - base)
        out_a[base:base + cnt] = res.results[c]["out_a"][:cnt].astype(np.float32)
        out_b[base:base + cnt] = res.results[c]["out_b"][:cnt].astype(np.float32)
    return out_a, out_b
